# revision 9
# baseline (speedup 1.0000x reference)
"""Bass/Trainium2 kernel for a fused GRU cell.

  r   = sigmoid(x @ W_ir.T + h @ W_hr.T + b_r)
  z   = sigmoid(x @ W_iz.T + h @ W_hz.T + b_z)
  g   = tanh  (x @ W_ih.T + (r*h) @ W_hh.T + b_h)
  h_t = (1-z)*h + z*g

Sharding: data-parallel over the batch (8192 -> 1024 rows per core on 8
NeuronCores), weights replicated, no collectives.

Mixed precision (validated against the 2e-2 rel-err budget, ~1e-2 achieved):
  - r gate: fp8 e4m3 DoubleRow matmuls (2x PE rate). Weights pre-scaled by
    64 on host so they sit in e4m3's normal range; the 1/64 is folded into
    the sigmoid's scale operand. r's quantization error washes out through
    the (r*h) @ W_hh contraction, unlike z / h-tilde whose errors hit the
    output directly -- those two run in bf16.
  - z, h-tilde gates: bf16 weights and moving operands (fp32 PSUM accum).
  - h_t is stored bf16 and upcast on host.

Layout is transposed ([hidden, batch], hidden on SBUF partitions) so biases
are per-partition scalars and all DMAs are contiguous. Input tiles are
DMA'd in 64KB chunks across queues so the first matmuls start ~4us in
instead of waiting on monolithic 512KB per-tile descriptors.
"""

import sys

for _p in ("/opt/trn_rl_repo", "/root/.axon_site/_ro/trn_rl_repo"):
    if _p not in sys.path:
        sys.path.append(_p)

import numpy as np

P = 128          # SBUF partitions
BC = 512         # PSUM bank free dim (fp32)
N_CORES = 8
S_R = 64.0       # r-gate fp8 weight prescale
QT = 12          # bf16 weight k-tiles per DMA slab
QR = 12          # fp8 weight k-pair-tiles per DMA slab (one h-tile per slab)

_PROG_CACHE = {}


def build_program(Bc, IN, H):
    """Build the per-core SPMD Bass program (identical on all cores)."""
    from contextlib import ExitStack

    from concourse import bacc, bass, mybir, tile
    from concourse.dt import dt

    KI, KH, NT = IN // P, H // P, H // P
    NJ = KI + KH                 # bf16 contraction tiles per gate per h-tile
    NJP = NJ // 2                # fp8 DoubleRow pair-tiles (KI even)
    NQ = NJ // QT                # bf16 slabs per gate per h-tile
    NQR = NJP // QR              # fp8 slabs per h-tile
    NB = Bc // BC
    f32, bf16, f8 = dt.float32, dt.bfloat16, dt.float8e4
    SIG = mybir.ActivationFunctionType.Sigmoid
    TANH = mybir.ActivationFunctionType.Tanh
    DR = mybir.MatmulPerfMode.DoubleRow

    nc = bacc.Bacc("TRN2", debug=False)
    x8_d = nc.declare_dram_parameter("x8", [P, KI, Bc], f8, False)
    h8_d = nc.declare_dram_parameter("h8", [P, KH, Bc], f8, False)
    xb_d = nc.declare_dram_parameter("xb", [P, KI, Bc], bf16, False)
    hb_d = nc.declare_dram_parameter("hb", [P, KH, Bc], bf16, False)
    wr_d = nc.declare_dram_parameter("wr", [NT, NQR, P, QR, 2, P], f8, False)
    wz_d = nc.declare_dram_parameter("wz", [NT, NQ, P, QT, P], bf16, False)
    wh_d = nc.declare_dram_parameter("wh", [NT, NQ, P, QT, P], bf16, False)
    b_d = nc.declare_dram_parameter("bias", [P, NT * 3], f32, False)
    out_d = nc.declare_dram_parameter("out", [NT, P, Bc], bf16, True)

    with ExitStack() as ctx:
        tc = ctx.enter_context(tile.TileContext(nc))
        res = ctx.enter_context(tc.tile_pool(name="res", bufs=1))
        wp = ctx.enter_context(tc.tile_pool(name="wp", bufs=6))
        pp = ctx.enter_context(
            tc.tile_pool(name="pp", bufs=4, space=bass.MemorySpace.PSUM)
        )
        op = ctx.enter_context(tc.tile_pool(name="op", bufs=2))
        zp = ctx.enter_context(tc.tile_pool(name="zp", bufs=2))

        x8 = res.tile([P, KI, Bc], f8, tag="x8")
        h8 = res.tile([P, KH, Bc], f8, tag="h8")
        xb = res.tile([P, KI, Bc], bf16, tag="xb")
        hb = res.tile([P, KH, Bc], bf16, tag="hb")
        rhb = res.tile([P, KH, Bc], bf16, tag="rhb")
        # all r-gate fp8 weights stay resident (48KB/partition) so their
        # DMA triggers need no pool flow control and can all fire up front
        wr_all = res.tile([P, NT * NJP, 2, P], f8, tag="wr")
        bias = res.tile([P, NT * 3], f32, tag="bias")

        # All input loads on the sync queue in exact consumption order: the
        # DMA rings drain FIFO, so this ordering IS the data-arrival order.
        def wr_slab(hti):
            for q in range(NQR):
                o = hti * NJP + q * QR
                nc.sync.dma_start(out=wr_all[:, o : o + QR], in_=wr_d[hti, q])

        # one dma_start per tensor => 128 descriptors of KI*Bc..KH*Bc bytes
        # each (per-partition rows), instead of per-k-tile 1KB descriptors
        # that run the rings at ~1/3 rate.
        nc.sync.dma_start(out=bias[:], in_=b_d[:])
        nc.sync.dma_start(out=x8[:], in_=x8_d[:])
        wr_slab(0)
        nc.sync.dma_start(out=h8[:], in_=h8_d[:])
        for hti in range(1, NT):
            wr_slab(hti)
        # bf16 inputs (phase ZH): behind all phase-R data, ahead of ZH slabs
        nc.sync.dma_start(out=xb[:], in_=xb_d[:])
        nc.sync.dma_start(out=hb[:], in_=hb_d[:])

        # ---- phase R: r = sigmoid((gi_r + gh_r)/S + b_r); rhb = r * h ----
        for hti in range(NT):
            ps = pp.tile([P, Bc], f32, tag="ps")
            for pj in range(NJP):
                mov = (
                    x8[:, 2 * pj : 2 * pj + 2, :]
                    if pj < KI // 2
                    else h8[:, 2 * pj - KI : 2 * pj - KI + 2, :]
                )
                for bc in range(NB):
                    sl = slice(bc * BC, (bc + 1) * BC)
                    nc.tensor.matmul(
                        ps[:, sl],
                        wr_all[:, hti * NJP + pj],
                        mov[:, :, sl],
                        start=(pj == 0),
                        stop=(pj == NJP - 1),
                        perf_mode=DR,
                        skip_group_check=True,
                    )
            for bc in range(NB):
                sl = slice(bc * BC, (bc + 1) * BC)
                nc.scalar.activation(
                    ps[:, sl], ps[:, sl], SIG,
                    bias=bias[:, hti * 3 : hti * 3 + 1], scale=1.0 / S_R,
                )
                nc.vector.tensor_mul(rhb[:, hti, sl], ps[:, sl], hb[:, hti, sl])

        def gate(ps, w_d, hti, srch):
            # ps[:, bc] += sum_j W_tile[j].T @ moving[j][:, bc]   (bf16)
            for q in range(NQ):
                slab = wp.tile([P, QT, P], bf16, tag="w")
                nc.sync.dma_start(out=slab[:], in_=w_d[hti, q])
                for jj in range(QT):
                    j = q * QT + jj
                    mov = xb[:, j, :] if j < KI else srch[:, j - KI, :]
                    for bc in range(NB):
                        sl = slice(bc * BC, (bc + 1) * BC)
                        nc.tensor.matmul(
                            ps[:, sl],
                            slab[:, jj],
                            mov[:, sl],
                            start=(j == 0),
                            stop=(j == NJ - 1),
                            skip_group_check=True,
                        )

        # ---- phase ZH: z, g, h_t = h + z*(g - h) ----
        for hti in range(NT):
            psz = pp.tile([P, Bc], f32, tag="ps")
            gate(psz, wz_d, hti, hb)
            psh = pp.tile([P, Bc], f32, tag="ps")
            gate(psh, wh_d, hti, rhb)
            for bc in range(NB):
                sl = slice(bc * BC, (bc + 1) * BC)
                # z straight into SBUF (DVE may read only one PSUM operand)
                zs = zp.tile([P, BC], f32, tag="zs")
                nc.scalar.activation(
                    zs[:], psz[:, sl], SIG,
                    bias=bias[:, hti * 3 + 1 : hti * 3 + 2],
                )
                nc.scalar.activation(
                    psh[:, sl], psh[:, sl], TANH,
                    bias=bias[:, hti * 3 + 2 : hti * 3 + 3],
                )
                nc.vector.tensor_sub(psh[:, sl], psh[:, sl], hb[:, hti, sl])
                nc.vector.tensor_mul(psh[:, sl], zs[:], psh[:, sl])
                o = op.tile([P, BC], bf16, tag="o")
                nc.vector.tensor_add(o[:], psh[:, sl], hb[:, hti, sl])
                nc.gpsimd.dma_start(out=out_d[hti, :, sl], in_=o[:])

    nc.compile()
    return nc


def _to_e4m3(a):
    import ml_dtypes

    return np.clip(a, -240.0, 240.0).astype(ml_dtypes.float8_e4m3)


def _to_bf16(a):
    import ml_dtypes

    return a.astype(ml_dtypes.bfloat16)


def _tiles_cat(Wi, Wh):
    """Stack [Wi-tiles; Wh-tiles] -> (NT, NJ, p, m) of 128x128 W.T blocks.

    cat[hti, j][p, m] = W[hti*P + m, k] with k = j*P + p.
    """
    H, IN = Wi.shape
    KI, KH, NT = IN // P, H // P, H // P
    ti = Wi.reshape(NT, P, KI, P).transpose(0, 2, 3, 1)
    th = Wh.reshape(NT, P, KH, P).transpose(0, 2, 3, 1)
    return np.concatenate([ti, th], axis=1)


def _pack_w_bf16(Wi, Wh):
    """-> (NT, NQ, P, QT, P) bf16 DMA-slab layout."""
    cat = _tiles_cat(Wi, Wh)                       # (NT, NJ, p, m)
    NT, NJ = cat.shape[:2]
    NQ = NJ // QT
    return np.ascontiguousarray(
        _to_bf16(cat.reshape(NT, NQ, QT, P, P).transpose(0, 1, 3, 2, 4))
    )


def _pack_w_fp8(Wi, Wh):
    """-> (NT, NQR, P, QR, 2, P) e4m3 DoubleRow pair-slab layout, x S_R."""
    cat = _tiles_cat(Wi, Wh) * S_R
    NT, NJ = cat.shape[:2]
    NQR = NJ // 2 // QR
    return np.ascontiguousarray(
        _to_e4m3(
            cat.reshape(NT, NQR, QR, 2, P, P).transpose(0, 1, 4, 2, 3, 5)
        )
    )


def _pack_acts(a):
    """(Bc, D) -> (P, D//P, Bc) with [p, t, b] = a[b, t*P + p]."""
    Bc, D = a.shape
    return np.ascontiguousarray(a.T.reshape(D // P, P, Bc).transpose(1, 0, 2))


def run(x_t, h_prev, W_ir, W_iz, W_ih, W_hr, W_hz, W_hh, b_r, b_z, b_h,
        trace=False):
    from concourse.bass_utils import run_bass_kernel_spmd

    x_t = np.asarray(x_t, dtype=np.float32)
    h_prev = np.asarray(h_prev, dtype=np.float32)
    B, IN = x_t.shape
    H = h_prev.shape[1]
    assert B % N_CORES == 0
    Bc = B // N_CORES
    NT = H // P

    key = (Bc, IN, H)
    if key not in _PROG_CACHE:
        _PROG_CACHE[key] = build_program(Bc, IN, H)
    nc = _PROG_CACHE[key]

    wr = _pack_w_fp8(np.asarray(W_ir, np.float32), np.asarray(W_hr, np.float32))
    wz = _pack_w_bf16(np.asarray(W_iz, np.float32), np.asarray(W_hz, np.float32))
    wh = _pack_w_bf16(np.asarray(W_ih, np.float32), np.asarray(W_hh, np.float32))
    bias = np.ascontiguousarray(
        np.stack(
            [np.asarray(b_r, np.float32), np.asarray(b_z, np.float32),
             np.asarray(b_h, np.float32)], axis=-1
        ).reshape(NT, P, 3).transpose(1, 0, 2).reshape(P, NT * 3)
    )

    in_maps = []
    for c in range(N_CORES):
        rows = slice(c * Bc, (c + 1) * Bc)
        xp = _pack_acts(x_t[rows])
        hp = _pack_acts(h_prev[rows])
        in_maps.append({
            "x8": _to_e4m3(xp), "h8": _to_e4m3(hp),
            "xb": _to_bf16(xp), "hb": _to_bf16(hp),
            "wr": wr, "wz": wz, "wh": wh, "bias": bias,
        })

    kw = {}
    if trace:
        kw = dict(trace=True, trace_cores=[0])
    res = run_bass_kernel_spmd(nc, in_maps, core_ids=list(range(N_CORES)), **kw)

    outs = []
    for c in range(N_CORES):
        o = np.asarray(res.results[c]["out"]).astype(np.float32)  # (NT, P, Bc)
        outs.append(o.reshape(H, Bc).T)                           # (Bc, H)
    full = np.concatenate(outs, axis=0).astype(np.float32)
    return (full, res) if trace else full


def kernel(**inputs):
    return run(**inputs)


# revision 10
# speedup vs baseline: 1.0272x; 1.0272x over previous
"""Bass/Trainium2 kernel for a fused GRU cell.

  r   = sigmoid(x @ W_ir.T + h @ W_hr.T + b_r)
  z   = sigmoid(x @ W_iz.T + h @ W_hz.T + b_z)
  g   = tanh  (x @ W_ih.T + (r*h) @ W_hh.T + b_h)
  h_t = (1-z)*h + z*g

Sharding: data-parallel over the batch (8192 -> 1024 rows per core on 8
NeuronCores), weights replicated, no collectives.

Mixed precision (validated against the 2e-2 rel-err budget, ~1e-2 achieved):
  - r gate: fp8 e4m3 DoubleRow matmuls (2x PE rate). Weights pre-scaled by
    64 on host so they sit in e4m3's normal range; the 1/64 is folded into
    the sigmoid's scale operand. r's quantization error washes out through
    the (r*h) @ W_hh contraction, unlike z / h-tilde whose errors hit the
    output directly -- those two run in bf16.
  - z, h-tilde gates: bf16 weights and moving operands (fp32 PSUM accum).
  - h_t is stored bf16 and upcast on host.

Layout is transposed ([hidden, batch], hidden on SBUF partitions) so biases
are per-partition scalars and all DMAs are contiguous. Input tiles are
DMA'd in 64KB chunks across queues so the first matmuls start ~4us in
instead of waiting on monolithic 512KB per-tile descriptors.
"""

import sys

for _p in ("/opt/trn_rl_repo", "/root/.axon_site/_ro/trn_rl_repo"):
    if _p not in sys.path:
        sys.path.append(_p)

import numpy as np

P = 128          # SBUF partitions
BC = 512         # PSUM bank free dim (fp32)
N_CORES = 8
S_R = 64.0       # r-gate fp8 weight prescale
QT = 12          # bf16 weight k-tiles per DMA slab
QR = 12          # fp8 weight k-pair-tiles per DMA slab (one h-tile per slab)

_PROG_CACHE = {}


def build_program(Bc, IN, H):
    """Build the per-core SPMD Bass program (identical on all cores)."""
    from contextlib import ExitStack

    from concourse import bacc, bass, mybir, tile
    from concourse.dt import dt

    KI, KH, NT = IN // P, H // P, H // P
    NJ = KI + KH                 # bf16 contraction tiles per gate per h-tile
    NJP = NJ // 2                # fp8 DoubleRow pair-tiles (KI even)
    NQ = NJ // QT                # bf16 slabs per gate per h-tile
    NQR = NJP // QR              # fp8 slabs per h-tile
    NB = Bc // BC
    f32, bf16, f8 = dt.float32, dt.bfloat16, dt.float8e4
    SIG = mybir.ActivationFunctionType.Sigmoid
    TANH = mybir.ActivationFunctionType.Tanh
    DR = mybir.MatmulPerfMode.DoubleRow

    nc = bacc.Bacc("TRN2", debug=False)
    x8_d = nc.declare_dram_parameter("x8", [P, KI, Bc], f8, False)
    h8_d = nc.declare_dram_parameter("h8", [P, KH, Bc], f8, False)
    xb_d = nc.declare_dram_parameter("xb", [P, KI, Bc], bf16, False)
    hb_d = nc.declare_dram_parameter("hb", [P, KH, Bc], bf16, False)
    wr_d = nc.declare_dram_parameter("wr", [NT, NQR, P, QR, 2, P], f8, False)
    wz_d = nc.declare_dram_parameter("wz", [NT, NQ, P, QT, P], bf16, False)
    wh_d = nc.declare_dram_parameter("wh", [NT, NQ, P, QT, P], bf16, False)
    b_d = nc.declare_dram_parameter("bias", [P, NT * 3], f32, False)
    out_d = nc.declare_dram_parameter("out", [NT, P, Bc], bf16, True)

    with ExitStack() as ctx:
        tc = ctx.enter_context(tile.TileContext(nc))
        res = ctx.enter_context(tc.tile_pool(name="res", bufs=1))
        wp = ctx.enter_context(tc.tile_pool(name="wp", bufs=6))
        pp = ctx.enter_context(
            tc.tile_pool(name="pp", bufs=4, space=bass.MemorySpace.PSUM)
        )
        op = ctx.enter_context(tc.tile_pool(name="op", bufs=2))
        zp = ctx.enter_context(tc.tile_pool(name="zp", bufs=2))

        x8 = res.tile([P, KI, Bc], f8, tag="x8")
        h8 = res.tile([P, KH, Bc], f8, tag="h8")
        xb = res.tile([P, KI, Bc], bf16, tag="xb")
        hb = res.tile([P, KH, Bc], bf16, tag="hb")
        rhb = res.tile([P, KH, Bc], bf16, tag="rhb")
        # all r-gate fp8 weights stay resident (48KB/partition) so their
        # DMA triggers need no pool flow control and can all fire up front
        wr_all = res.tile([P, NT * NJP, 2, P], f8, tag="wr")
        bias = res.tile([P, NT * 3], f32, tag="bias")

        # All input loads on the sync queue in exact consumption order: the
        # DMA rings drain FIFO, so this ordering IS the data-arrival order.
        def wr_slab(hti):
            for q in range(NQR):
                o = hti * NJP + q * QR
                nc.sync.dma_start(out=wr_all[:, o : o + QR], in_=wr_d[hti, q])

        # Trigger instructions (DIRECT2D) cost ~700ns each on the issuing
        # sequencer, so batch tensors into few dma_starts — but only where
        # the first consumer needs the whole tensor anyway (x8/h8/xb).
        # hb MUST stay chunked: the per-h-tile rhb muls (which recycle the
        # PSUM pool that gates phase R) consume it tile by tile.
        nc.sync.dma_start(out=bias[:], in_=b_d[:])
        nc.sync.dma_start(out=x8[:, : KI // 2], in_=x8_d[:, : KI // 2])
        wr_slab(0)
        nc.sync.dma_start(out=x8[:, KI // 2 :], in_=x8_d[:, KI // 2 :])
        nc.sync.dma_start(out=h8[:], in_=h8_d[:])
        for t in range(0, KH, 2):
            nc.sync.dma_start(out=hb[:, t : t + 2, :], in_=hb_d[:, t : t + 2])
        for hti in range(1, NT):
            wr_slab(hti)
        # bf16 x (phase ZH): behind all phase-R data, ahead of ZH slabs
        nc.sync.dma_start(out=xb[:], in_=xb_d[:])

        # ---- phase R: r = sigmoid((gi_r + gh_r)/S + b_r); rhb = r * h ----
        for hti in range(NT):
            ps = pp.tile([P, Bc], f32, tag="ps")
            for pj in range(NJP):
                mov = (
                    x8[:, 2 * pj : 2 * pj + 2, :]
                    if pj < KI // 2
                    else h8[:, 2 * pj - KI : 2 * pj - KI + 2, :]
                )
                for bc in range(NB):
                    sl = slice(bc * BC, (bc + 1) * BC)
                    nc.tensor.matmul(
                        ps[:, sl],
                        wr_all[:, hti * NJP + pj],
                        mov[:, :, sl],
                        start=(pj == 0),
                        stop=(pj == NJP - 1),
                        perf_mode=DR,
                        skip_group_check=True,
                    )
            for bc in range(NB):
                sl = slice(bc * BC, (bc + 1) * BC)
                nc.scalar.activation(
                    ps[:, sl], ps[:, sl], SIG,
                    bias=bias[:, hti * 3 : hti * 3 + 1], scale=1.0 / S_R,
                )
                nc.vector.tensor_mul(rhb[:, hti, sl], ps[:, sl], hb[:, hti, sl])

        def gate(ps, w_d, hti, srch):
            # ps[:, bc] += sum_j W_tile[j].T @ moving[j][:, bc]   (bf16)
            for q in range(NQ):
                slab = wp.tile([P, QT, P], bf16, tag="w")
                nc.sync.dma_start(out=slab[:], in_=w_d[hti, q])
                for jj in range(QT):
                    j = q * QT + jj
                    mov = xb[:, j, :] if j < KI else srch[:, j - KI, :]
                    for bc in range(NB):
                        sl = slice(bc * BC, (bc + 1) * BC)
                        nc.tensor.matmul(
                            ps[:, sl],
                            slab[:, jj],
                            mov[:, sl],
                            start=(j == 0),
                            stop=(j == NJ - 1),
                            skip_group_check=True,
                        )

        # ---- phase ZH: z, g, h_t = h + z*(g - h) ----
        for hti in range(NT):
            psz = pp.tile([P, Bc], f32, tag="ps")
            gate(psz, wz_d, hti, hb)
            psh = pp.tile([P, Bc], f32, tag="ps")
            gate(psh, wh_d, hti, rhb)
            for bc in range(NB):
                sl = slice(bc * BC, (bc + 1) * BC)
                # z straight into SBUF (DVE may read only one PSUM operand)
                zs = zp.tile([P, BC], f32, tag="zs")
                nc.scalar.activation(
                    zs[:], psz[:, sl], SIG,
                    bias=bias[:, hti * 3 + 1 : hti * 3 + 2],
                )
                nc.scalar.activation(
                    psh[:, sl], psh[:, sl], TANH,
                    bias=bias[:, hti * 3 + 2 : hti * 3 + 3],
                )
                nc.vector.tensor_sub(psh[:, sl], psh[:, sl], hb[:, hti, sl])
                nc.vector.tensor_mul(psh[:, sl], zs[:], psh[:, sl])
                o = op.tile([P, BC], bf16, tag="o")
                nc.vector.tensor_add(o[:], psh[:, sl], hb[:, hti, sl])
                nc.gpsimd.dma_start(out=out_d[hti, :, sl], in_=o[:])

    nc.compile()
    return nc


def _to_e4m3(a):
    import ml_dtypes

    return np.clip(a, -240.0, 240.0).astype(ml_dtypes.float8_e4m3)


def _to_bf16(a):
    import ml_dtypes

    return a.astype(ml_dtypes.bfloat16)


def _tiles_cat(Wi, Wh):
    """Stack [Wi-tiles; Wh-tiles] -> (NT, NJ, p, m) of 128x128 W.T blocks.

    cat[hti, j][p, m] = W[hti*P + m, k] with k = j*P + p.
    """
    H, IN = Wi.shape
    KI, KH, NT = IN // P, H // P, H // P
    ti = Wi.reshape(NT, P, KI, P).transpose(0, 2, 3, 1)
    th = Wh.reshape(NT, P, KH, P).transpose(0, 2, 3, 1)
    return np.concatenate([ti, th], axis=1)


def _pack_w_bf16(Wi, Wh):
    """-> (NT, NQ, P, QT, P) bf16 DMA-slab layout."""
    cat = _tiles_cat(Wi, Wh)                       # (NT, NJ, p, m)
    NT, NJ = cat.shape[:2]
    NQ = NJ // QT
    return np.ascontiguousarray(
        _to_bf16(cat.reshape(NT, NQ, QT, P, P).transpose(0, 1, 3, 2, 4))
    )


def _pack_w_fp8(Wi, Wh):
    """-> (NT, NQR, P, QR, 2, P) e4m3 DoubleRow pair-slab layout, x S_R."""
    cat = _tiles_cat(Wi, Wh) * S_R
    NT, NJ = cat.shape[:2]
    NQR = NJ // 2 // QR
    return np.ascontiguousarray(
        _to_e4m3(
            cat.reshape(NT, NQR, QR, 2, P, P).transpose(0, 1, 4, 2, 3, 5)
        )
    )


def _pack_acts(a):
    """(Bc, D) -> (P, D//P, Bc) with [p, t, b] = a[b, t*P + p]."""
    Bc, D = a.shape
    return np.ascontiguousarray(a.T.reshape(D // P, P, Bc).transpose(1, 0, 2))


def run(x_t, h_prev, W_ir, W_iz, W_ih, W_hr, W_hz, W_hh, b_r, b_z, b_h,
        trace=False):
    from concourse.bass_utils import run_bass_kernel_spmd

    x_t = np.asarray(x_t, dtype=np.float32)
    h_prev = np.asarray(h_prev, dtype=np.float32)
    B, IN = x_t.shape
    H = h_prev.shape[1]
    assert B % N_CORES == 0
    Bc = B // N_CORES
    NT = H // P

    key = (Bc, IN, H)
    if key not in _PROG_CACHE:
        _PROG_CACHE[key] = build_program(Bc, IN, H)
    nc = _PROG_CACHE[key]

    wr = _pack_w_fp8(np.asarray(W_ir, np.float32), np.asarray(W_hr, np.float32))
    wz = _pack_w_bf16(np.asarray(W_iz, np.float32), np.asarray(W_hz, np.float32))
    wh = _pack_w_bf16(np.asarray(W_ih, np.float32), np.asarray(W_hh, np.float32))
    bias = np.ascontiguousarray(
        np.stack(
            [np.asarray(b_r, np.float32), np.asarray(b_z, np.float32),
             np.asarray(b_h, np.float32)], axis=-1
        ).reshape(NT, P, 3).transpose(1, 0, 2).reshape(P, NT * 3)
    )

    in_maps = []
    for c in range(N_CORES):
        rows = slice(c * Bc, (c + 1) * Bc)
        xp = _pack_acts(x_t[rows])
        hp = _pack_acts(h_prev[rows])
        in_maps.append({
            "x8": _to_e4m3(xp), "h8": _to_e4m3(hp),
            "xb": _to_bf16(xp), "hb": _to_bf16(hp),
            "wr": wr, "wz": wz, "wh": wh, "bias": bias,
        })

    kw = {}
    if trace:
        kw = dict(trace=True, trace_cores=[0])
    res = run_bass_kernel_spmd(nc, in_maps, core_ids=list(range(N_CORES)), **kw)

    outs = []
    for c in range(N_CORES):
        o = np.asarray(res.results[c]["out"]).astype(np.float32)  # (NT, P, Bc)
        outs.append(o.reshape(H, Bc).T)                           # (Bc, H)
    full = np.concatenate(outs, axis=0).astype(np.float32)
    return (full, res) if trace else full


def kernel(**inputs):
    return run(**inputs)


# revision 11
# speedup vs baseline: 1.0473x; 1.0195x over previous
"""Bass/Trainium2 kernel for a fused GRU cell.

  r   = sigmoid(x @ W_ir.T + h @ W_hr.T + b_r)
  z   = sigmoid(x @ W_iz.T + h @ W_hz.T + b_z)
  g   = tanh  (x @ W_ih.T + (r*h) @ W_hh.T + b_h)
  h_t = (1-z)*h + z*g

Sharding: data-parallel over the batch (8192 -> 1024 rows per core on 8
NeuronCores), weights replicated, no collectives.

Mixed precision (validated against the 2e-2 rel-err budget, ~1e-2 achieved):
  - r gate: fp8 e4m3 DoubleRow matmuls (2x PE rate). Weights pre-scaled by
    64 on host so they sit in e4m3's normal range; the 1/64 is folded into
    the sigmoid's scale operand. r's quantization error washes out through
    the (r*h) @ W_hh contraction, unlike z / h-tilde whose errors hit the
    output directly -- those two run in bf16.
  - z, h-tilde gates: bf16 weights and moving operands (fp32 PSUM accum).
  - h_t is stored bf16 and upcast on host.

Layout is transposed ([hidden, batch], hidden on SBUF partitions) so biases
are per-partition scalars and all DMAs are contiguous. Input tiles are
DMA'd in 64KB chunks across queues so the first matmuls start ~4us in
instead of waiting on monolithic 512KB per-tile descriptors.
"""

import sys

for _p in ("/opt/trn_rl_repo", "/root/.axon_site/_ro/trn_rl_repo"):
    if _p not in sys.path:
        sys.path.append(_p)

import numpy as np

P = 128          # SBUF partitions
BC = 512         # PSUM bank free dim (fp32)
N_CORES = 8
S_R = 64.0       # r-gate fp8 weight prescale
QT = 12          # bf16 weight k-tiles per DMA slab
QR = 12          # fp8 weight k-pair-tiles per DMA slab (one h-tile per slab)

_PROG_CACHE = {}


def build_program(Bc, IN, H):
    """Build the per-core SPMD Bass program (identical on all cores)."""
    from contextlib import ExitStack

    from concourse import bacc, bass, mybir, tile
    from concourse.dt import dt

    KI, KH, NT = IN // P, H // P, H // P
    NJ = KI + KH                 # bf16 contraction tiles per gate per h-tile
    NJP = NJ // 2                # fp8 DoubleRow pair-tiles (KI even)
    NQ = NJ // QT                # bf16 slabs per gate per h-tile
    NQR = NJP // QR              # fp8 slabs per h-tile
    NB = Bc // BC
    f32, bf16, f8 = dt.float32, dt.bfloat16, dt.float8e4
    SIG = mybir.ActivationFunctionType.Sigmoid
    TANH = mybir.ActivationFunctionType.Tanh
    DR = mybir.MatmulPerfMode.DoubleRow

    nc = bacc.Bacc("TRN2", debug=False)
    x8_d = nc.declare_dram_parameter("x8", [P, KI, Bc], f8, False)
    h8_d = nc.declare_dram_parameter("h8", [P, KH, Bc], f8, False)
    xb_d = nc.declare_dram_parameter("xb", [P, KI, Bc], bf16, False)
    hb_d = nc.declare_dram_parameter("hb", [P, KH, Bc], bf16, False)
    wr_d = nc.declare_dram_parameter("wr", [NT, NQR, P, QR, 2, P], f8, False)
    wz_d = nc.declare_dram_parameter("wz", [NT, NQ, P, QT, P], bf16, False)
    wh_d = nc.declare_dram_parameter("wh", [NT, NQ, P, QT, P], bf16, False)
    b_d = nc.declare_dram_parameter("bias", [P, NT * 3], f32, False)
    out_d = nc.declare_dram_parameter("out", [NT, P, Bc], bf16, True)

    with ExitStack() as ctx:
        tc = ctx.enter_context(tile.TileContext(nc))
        res = ctx.enter_context(tc.tile_pool(name="res", bufs=1))
        wp = ctx.enter_context(tc.tile_pool(name="wp", bufs=6))
        pp = ctx.enter_context(
            tc.tile_pool(name="pp", bufs=4, space=bass.MemorySpace.PSUM)
        )
        op = ctx.enter_context(tc.tile_pool(name="op", bufs=2))
        zp = ctx.enter_context(tc.tile_pool(name="zp", bufs=2))

        x8 = res.tile([P, KI, Bc], f8, tag="x8")
        h8 = res.tile([P, KH, Bc], f8, tag="h8")
        xb = res.tile([P, KI, Bc], bf16, tag="xb")
        hb = res.tile([P, KH, Bc], bf16, tag="hb")
        rhb = res.tile([P, KH, Bc], bf16, tag="rhb")
        # all r-gate fp8 weights stay resident (48KB/partition) so their
        # DMA triggers need no pool flow control and can all fire up front
        wr_all = res.tile([P, NT * NJP, 2, P], f8, tag="wr")
        bias = res.tile([P, NT * 3], f32, tag="bias")

        # All input loads on the sync queue in exact consumption order: the
        # DMA rings drain FIFO, so this ordering IS the data-arrival order.
        def wr_slab(hti):
            for q in range(NQR):
                o = hti * NJP + q * QR
                nc.sync.dma_start(out=wr_all[:, o : o + QR], in_=wr_d[hti, q])

        # Trigger instructions (DIRECT2D) cost ~700ns each on the issuing
        # sequencer, so batch tensors into few dma_starts — but only where
        # the first consumer needs the whole tensor anyway (x8/h8/xb).
        # hb MUST stay chunked: the per-h-tile rhb muls (which recycle the
        # PSUM pool that gates phase R) consume it tile by tile.
        nc.sync.dma_start(out=bias[:], in_=b_d[:])
        nc.sync.dma_start(out=x8[:, : KI // 2], in_=x8_d[:, : KI // 2])
        wr_slab(0)
        nc.sync.dma_start(out=x8[:, KI // 2 :], in_=x8_d[:, KI // 2 :])
        nc.sync.dma_start(out=h8[:], in_=h8_d[:])
        # interleave r slabs (consumed 1 per 5.2us) with hb chunks (1 per
        # 10.3us via the rhb muls) so neither starves the other
        for i in range(1, NT):
            wr_slab(i)
            if i <= KH // 2:
                t = 2 * (i - 1)
                nc.sync.dma_start(out=hb[:, t : t + 2, :], in_=hb_d[:, t : t + 2])
        # bf16 x (phase ZH): behind all phase-R data, ahead of ZH slabs
        nc.sync.dma_start(out=xb[:], in_=xb_d[:])

        # ---- phase R: r = sigmoid((gi_r + gh_r)/S + b_r); rhb = r * h ----
        for hti in range(NT):
            ps = pp.tile([P, Bc], f32, tag="ps")
            for pj in range(NJP):
                mov = (
                    x8[:, 2 * pj : 2 * pj + 2, :]
                    if pj < KI // 2
                    else h8[:, 2 * pj - KI : 2 * pj - KI + 2, :]
                )
                for bc in range(NB):
                    sl = slice(bc * BC, (bc + 1) * BC)
                    nc.tensor.matmul(
                        ps[:, sl],
                        wr_all[:, hti * NJP + pj],
                        mov[:, :, sl],
                        start=(pj == 0),
                        stop=(pj == NJP - 1),
                        perf_mode=DR,
                        skip_group_check=True,
                    )
            for bc in range(NB):
                sl = slice(bc * BC, (bc + 1) * BC)
                nc.scalar.activation(
                    ps[:, sl], ps[:, sl], SIG,
                    bias=bias[:, hti * 3 : hti * 3 + 1], scale=1.0 / S_R,
                )
                nc.vector.tensor_mul(rhb[:, hti, sl], ps[:, sl], hb[:, hti, sl])

        def gate(ps, w_d, hti, srch):
            # ps[:, bc] += sum_j W_tile[j].T @ moving[j][:, bc]   (bf16)
            for q in range(NQ):
                slab = wp.tile([P, QT, P], bf16, tag="w")
                nc.sync.dma_start(out=slab[:], in_=w_d[hti, q])
                for jj in range(QT):
                    j = q * QT + jj
                    mov = xb[:, j, :] if j < KI else srch[:, j - KI, :]
                    for bc in range(NB):
                        sl = slice(bc * BC, (bc + 1) * BC)
                        nc.tensor.matmul(
                            ps[:, sl],
                            slab[:, jj],
                            mov[:, sl],
                            start=(j == 0),
                            stop=(j == NJ - 1),
                            skip_group_check=True,
                        )

        # ---- phase ZH: z, g, h_t = h + z*(g - h) ----
        for hti in range(NT):
            psz = pp.tile([P, Bc], f32, tag="ps")
            gate(psz, wz_d, hti, hb)
            psh = pp.tile([P, Bc], f32, tag="ps")
            gate(psh, wh_d, hti, rhb)
            for bc in range(NB):
                sl = slice(bc * BC, (bc + 1) * BC)
                # z straight into SBUF (DVE may read only one PSUM operand)
                zs = zp.tile([P, BC], f32, tag="zs")
                nc.scalar.activation(
                    zs[:], psz[:, sl], SIG,
                    bias=bias[:, hti * 3 + 1 : hti * 3 + 2],
                )
                nc.scalar.activation(
                    psh[:, sl], psh[:, sl], TANH,
                    bias=bias[:, hti * 3 + 2 : hti * 3 + 3],
                )
                nc.vector.tensor_sub(psh[:, sl], psh[:, sl], hb[:, hti, sl])
                nc.vector.tensor_mul(psh[:, sl], zs[:], psh[:, sl])
                o = op.tile([P, BC], bf16, tag="o")
                nc.vector.tensor_add(o[:], psh[:, sl], hb[:, hti, sl])
                nc.gpsimd.dma_start(out=out_d[hti, :, sl], in_=o[:])

    nc.compile()
    return nc


def _to_e4m3(a):
    import ml_dtypes

    return np.clip(a, -240.0, 240.0).astype(ml_dtypes.float8_e4m3)


def _to_bf16(a):
    import ml_dtypes

    return a.astype(ml_dtypes.bfloat16)


def _tiles_cat(Wi, Wh):
    """Stack [Wi-tiles; Wh-tiles] -> (NT, NJ, p, m) of 128x128 W.T blocks.

    cat[hti, j][p, m] = W[hti*P + m, k] with k = j*P + p.
    """
    H, IN = Wi.shape
    KI, KH, NT = IN // P, H // P, H // P
    ti = Wi.reshape(NT, P, KI, P).transpose(0, 2, 3, 1)
    th = Wh.reshape(NT, P, KH, P).transpose(0, 2, 3, 1)
    return np.concatenate([ti, th], axis=1)


def _pack_w_bf16(Wi, Wh):
    """-> (NT, NQ, P, QT, P) bf16 DMA-slab layout."""
    cat = _tiles_cat(Wi, Wh)                       # (NT, NJ, p, m)
    NT, NJ = cat.shape[:2]
    NQ = NJ // QT
    return np.ascontiguousarray(
        _to_bf16(cat.reshape(NT, NQ, QT, P, P).transpose(0, 1, 3, 2, 4))
    )


def _pack_w_fp8(Wi, Wh):
    """-> (NT, NQR, P, QR, 2, P) e4m3 DoubleRow pair-slab layout, x S_R."""
    cat = _tiles_cat(Wi, Wh) * S_R
    NT, NJ = cat.shape[:2]
    NQR = NJ // 2 // QR
    return np.ascontiguousarray(
        _to_e4m3(
            cat.reshape(NT, NQR, QR, 2, P, P).transpose(0, 1, 4, 2, 3, 5)
        )
    )


def _pack_acts(a):
    """(Bc, D) -> (P, D//P, Bc) with [p, t, b] = a[b, t*P + p]."""
    Bc, D = a.shape
    return np.ascontiguousarray(a.T.reshape(D // P, P, Bc).transpose(1, 0, 2))


def run(x_t, h_prev, W_ir, W_iz, W_ih, W_hr, W_hz, W_hh, b_r, b_z, b_h,
        trace=False):
    from concourse.bass_utils import run_bass_kernel_spmd

    x_t = np.asarray(x_t, dtype=np.float32)
    h_prev = np.asarray(h_prev, dtype=np.float32)
    B, IN = x_t.shape
    H = h_prev.shape[1]
    assert B % N_CORES == 0
    Bc = B // N_CORES
    NT = H // P

    key = (Bc, IN, H)
    if key not in _PROG_CACHE:
        _PROG_CACHE[key] = build_program(Bc, IN, H)
    nc = _PROG_CACHE[key]

    wr = _pack_w_fp8(np.asarray(W_ir, np.float32), np.asarray(W_hr, np.float32))
    wz = _pack_w_bf16(np.asarray(W_iz, np.float32), np.asarray(W_hz, np.float32))
    wh = _pack_w_bf16(np.asarray(W_ih, np.float32), np.asarray(W_hh, np.float32))
    bias = np.ascontiguousarray(
        np.stack(
            [np.asarray(b_r, np.float32), np.asarray(b_z, np.float32),
             np.asarray(b_h, np.float32)], axis=-1
        ).reshape(NT, P, 3).transpose(1, 0, 2).reshape(P, NT * 3)
    )

    in_maps = []
    for c in range(N_CORES):
        rows = slice(c * Bc, (c + 1) * Bc)
        xp = _pack_acts(x_t[rows])
        hp = _pack_acts(h_prev[rows])
        in_maps.append({
            "x8": _to_e4m3(xp), "h8": _to_e4m3(hp),
            "xb": _to_bf16(xp), "hb": _to_bf16(hp),
            "wr": wr, "wz": wz, "wh": wh, "bias": bias,
        })

    kw = {}
    if trace:
        kw = dict(trace=True, trace_cores=[0])
    res = run_bass_kernel_spmd(nc, in_maps, core_ids=list(range(N_CORES)), **kw)

    outs = []
    for c in range(N_CORES):
        o = np.asarray(res.results[c]["out"]).astype(np.float32)  # (NT, P, Bc)
        outs.append(o.reshape(H, Bc).T)                           # (Bc, H)
    full = np.concatenate(outs, axis=0).astype(np.float32)
    return (full, res) if trace else full


def kernel(**inputs):
    return run(**inputs)


# revision 12
# speedup vs baseline: 1.0974x; 1.0478x over previous
"""Bass/Trainium2 kernel for a fused GRU cell.

  r   = sigmoid(x @ W_ir.T + h @ W_hr.T + b_r)
  z   = sigmoid(x @ W_iz.T + h @ W_hz.T + b_z)
  g   = tanh  (x @ W_ih.T + (r*h) @ W_hh.T + b_h)
  h_t = (1-z)*h + z*g

Sharding: data-parallel over the batch (8192 -> 1024 rows per core on 8
NeuronCores), weights replicated, no collectives.

Mixed precision (numpy-simulated exactly: rel err 1.35e-2 vs 2e-2 budget):
  - r gate: entirely fp8 e4m3 DoubleRow matmuls (2 k-tiles per 213ns MM =
    2x PE rate). r's quantization error washes out through the (r*h) @
    W_hh contraction, unlike z / h-tilde whose errors hit the output
    directly.
  - z gate: bf16 except the last h k-pair (tiles 14,15) in fp8-DR.
  - h-tilde: bf16 except the last two rh k-pairs (tiles 12..15) in fp8-DR.
  - ALL gate weights are pre-scaled by 64 on host (exact in bf16, puts the
    fp8 weights in e4m3's normal range); every activation applies
    scale=1/64. Biases stay unscaled (activation computes f(x*scale+b)).
  - h_t is stored bf16 and upcast on host.

Layout is transposed ([hidden, batch], hidden on SBUF partitions) so biases
are per-partition scalars and all DMAs are contiguous. All fp8 weights are
preloaded into a resident SBUF tile; DMA triggers cost ~700ns each on the
issuing sequencer, so loads are batched, ordered in exact consumption
order on the sync queue, and hb stays chunked because the per-h-tile rhb
muls (which recycle the PSUM pool gating phase R) consume it tile by tile.
"""

import sys

for _p in ("/opt/trn_rl_repo", "/root/.axon_site/_ro/trn_rl_repo"):
    if _p not in sys.path:
        sys.path.append(_p)

import numpy as np

P = 128          # SBUF partitions
BC = 512         # PSUM bank free dim (fp32)
N_CORES = 8
S_R = 64.0       # weight prescale (undone in activation scale)
QT = 12          # bf16 weight k-tiles per DMA slab
NPZ = 1          # fp8 k-pairs in the z gate (h-side tail)
NPH = 2          # fp8 k-pairs in the h-tilde gate (rh-side tail)

_PROG_CACHE = {}


def build_program(Bc, IN, H):
    """Build the per-core SPMD Bass program (identical on all cores)."""
    from contextlib import ExitStack

    from concourse import bacc, bass, mybir, tile
    from concourse.dt import dt

    KI, KH, NT = IN // P, H // P, H // P
    NJ = KI + KH                 # contraction k-tiles per gate per h-tile
    NJP = NJ // 2                # r-gate fp8 pair-tiles
    PAIRS = NJP + NPZ + NPH      # fp8 pair-tiles per h-tile slab (r + z + h)
    NJZ = NJ - 2 * NPZ           # bf16 k-tiles in z gate
    NJH = NJ - 2 * NPH           # bf16 k-tiles in h-tilde gate
    NRB = KH - 2 * NPH           # rh tiles kept in bf16
    NQ = NJ // QT                # bf16 slabs per gate per h-tile
    NB = Bc // BC
    f32, bf16, f8 = dt.float32, dt.bfloat16, dt.float8e4
    SIG = mybir.ActivationFunctionType.Sigmoid
    TANH = mybir.ActivationFunctionType.Tanh
    DR = mybir.MatmulPerfMode.DoubleRow

    nc = bacc.Bacc("TRN2", debug=False)
    x8_d = nc.declare_dram_parameter("x8", [P, KI, Bc], f8, False)
    h8_d = nc.declare_dram_parameter("h8", [P, KH, Bc], f8, False)
    xb_d = nc.declare_dram_parameter("xb", [P, KI, Bc], bf16, False)
    hb_d = nc.declare_dram_parameter("hb", [P, KH, Bc], bf16, False)
    wr_d = nc.declare_dram_parameter("wr", [NT, P, PAIRS, 2, P], f8, False)
    wz_d = nc.declare_dram_parameter("wz", [NT, NQ, P, QT, P], bf16, False)
    wh_d = nc.declare_dram_parameter("wh", [NT, NQ, P, QT, P], bf16, False)
    b_d = nc.declare_dram_parameter("bias", [P, NT * 3], f32, False)
    out_d = nc.declare_dram_parameter("out", [NT, P, Bc], bf16, True)

    with ExitStack() as ctx:
        tc = ctx.enter_context(tile.TileContext(nc))
        res = ctx.enter_context(tc.tile_pool(name="res", bufs=1))
        wp = ctx.enter_context(tc.tile_pool(name="wp", bufs=6))
        pp = ctx.enter_context(
            tc.tile_pool(name="pp", bufs=4, space=bass.MemorySpace.PSUM)
        )
        op = ctx.enter_context(tc.tile_pool(name="op", bufs=2))
        zp = ctx.enter_context(tc.tile_pool(name="zp", bufs=2))

        x8 = res.tile([P, KI, Bc], f8, tag="x8")
        h8 = res.tile([P, KH, Bc], f8, tag="h8")
        xb = res.tile([P, KI, Bc], bf16, tag="xb")
        hb = res.tile([P, KH, Bc], bf16, tag="hb")
        rhb = res.tile([P, NRB, Bc], bf16, tag="rhb")
        rh8 = res.tile([P, 2 * NPH, Bc], f8, tag="rh8")
        wr_all = res.tile([P, NT * PAIRS, 2, P], f8, tag="wr")
        bias = res.tile([P, NT * 3], f32, tag="bias")

        def wr_slab(hti):
            o = hti * PAIRS
            nc.sync.dma_start(out=wr_all[:, o : o + PAIRS], in_=wr_d[hti])

        nc.sync.dma_start(out=bias[:], in_=b_d[:])
        nc.sync.dma_start(out=x8[:, : KI // 2], in_=x8_d[:, : KI // 2])
        wr_slab(0)
        nc.sync.dma_start(out=x8[:, KI // 2 :], in_=x8_d[:, KI // 2 :])
        nc.sync.dma_start(out=h8[:], in_=h8_d[:])
        # interleave r slabs (consumed 1 per 5.2us) with hb chunks (1 per
        # 10.3us via the rhb muls) so neither starves the other
        for i in range(1, NT):
            wr_slab(i)
            if i <= KH // 2:
                t = 2 * (i - 1)
                nc.sync.dma_start(out=hb[:, t : t + 2, :], in_=hb_d[:, t : t + 2])
        # bf16 x (phase ZH): behind all phase-R data, ahead of ZH slabs
        nc.sync.dma_start(out=xb[:], in_=xb_d[:])

        # ---- phase R: r = sigmoid((gi_r + gh_r)/S + b_r); rh = r * h ----
        for hti in range(NT):
            ps = pp.tile([P, Bc], f32, tag="ps")
            for pj in range(NJP):
                mov = (
                    x8[:, 2 * pj : 2 * pj + 2, :]
                    if pj < KI // 2
                    else h8[:, 2 * pj - KI : 2 * pj - KI + 2, :]
                )
                for bc in range(NB):
                    sl = slice(bc * BC, (bc + 1) * BC)
                    nc.tensor.matmul(
                        ps[:, sl],
                        wr_all[:, hti * PAIRS + pj],
                        mov[:, :, sl],
                        start=(pj == 0),
                        stop=(pj == NJP - 1),
                        perf_mode=DR,
                        skip_group_check=True,
                    )
            for bc in range(NB):
                sl = slice(bc * BC, (bc + 1) * BC)
                nc.scalar.activation(
                    ps[:, sl], ps[:, sl], SIG,
                    bias=bias[:, hti * 3 : hti * 3 + 1], scale=1.0 / S_R,
                )
                if hti < NRB:
                    nc.vector.tensor_mul(rhb[:, hti, sl], ps[:, sl], hb[:, hti, sl])
                else:
                    nc.vector.tensor_mul(
                        rh8[:, hti - NRB, sl], ps[:, sl], hb[:, hti, sl]
                    )

        def gate(ps, w_d, hti, srch, njb, pair0, pairs_mov):
            # bf16 part: ps[:, bc] += sum_{j<njb} W_tile[j].T @ moving[j]
            for q in range(NQ):
                slab = wp.tile([P, QT, P], bf16, tag="w")
                nc.sync.dma_start(out=slab[:], in_=w_d[hti, q])
                for jj in range(QT):
                    j = q * QT + jj
                    if j >= njb:
                        break
                    mov = xb[:, j, :] if j < KI else srch[:, j - KI, :]
                    for bc in range(NB):
                        sl = slice(bc * BC, (bc + 1) * BC)
                        nc.tensor.matmul(
                            ps[:, sl],
                            slab[:, jj],
                            mov[:, sl],
                            start=(j == 0),
                            stop=False,
                            skip_group_check=True,
                        )
            # fp8-DR tail pairs (weights live in the resident wr_all slab)
            for i, pmov in enumerate(pairs_mov):
                for bc in range(NB):
                    sl = slice(bc * BC, (bc + 1) * BC)
                    nc.tensor.matmul(
                        ps[:, sl],
                        wr_all[:, hti * PAIRS + pair0 + i],
                        pmov[:, :, sl],
                        start=False,
                        stop=(i == len(pairs_mov) - 1),
                        perf_mode=DR,
                        skip_group_check=True,
                    )

        # ---- phase ZH: z, g, h_t = h + z*(g - h) ----
        for hti in range(NT):
            psz = pp.tile([P, Bc], f32, tag="ps")
            gate(psz, wz_d, hti, hb, NJZ, NJP,
                 [h8[:, KH - 2 * NPZ + 2 * i : KH - 2 * NPZ + 2 * i + 2, :]
                  for i in range(NPZ)])
            psh = pp.tile([P, Bc], f32, tag="ps")
            gate(psh, wh_d, hti, rhb, NJH, NJP + NPZ,
                 [rh8[:, 2 * i : 2 * i + 2, :] for i in range(NPH)])
            for bc in range(NB):
                sl = slice(bc * BC, (bc + 1) * BC)
                # z straight into SBUF (DVE may read only one PSUM operand)
                zs = zp.tile([P, BC], f32, tag="zs")
                nc.scalar.activation(
                    zs[:], psz[:, sl], SIG,
                    bias=bias[:, hti * 3 + 1 : hti * 3 + 2], scale=1.0 / S_R,
                )
                nc.scalar.activation(
                    psh[:, sl], psh[:, sl], TANH,
                    bias=bias[:, hti * 3 + 2 : hti * 3 + 3], scale=1.0 / S_R,
                )
                nc.vector.tensor_sub(psh[:, sl], psh[:, sl], hb[:, hti, sl])
                nc.vector.tensor_mul(psh[:, sl], zs[:], psh[:, sl])
                o = op.tile([P, BC], bf16, tag="o")
                nc.vector.tensor_add(o[:], psh[:, sl], hb[:, hti, sl])
                nc.gpsimd.dma_start(out=out_d[hti, :, sl], in_=o[:])

    nc.compile()
    return nc


def _to_e4m3(a):
    import ml_dtypes

    return np.clip(a, -240.0, 240.0).astype(ml_dtypes.float8_e4m3)


def _to_bf16(a):
    import ml_dtypes

    return a.astype(ml_dtypes.bfloat16)


def _w_tiles(W):
    """(H, K) -> (NT, K//P, p, m) of 128x128 W.T blocks.

    t[hti, j][p, m] = W[hti*P + m, j*P + p]
    """
    H, K = W.shape
    return W.reshape(H // P, P, K // P, P).transpose(0, 2, 3, 1)


def _pack_w_bf16(Wi, Wh):
    """-> (NT, NQ, P, QT, P) bf16 DMA-slab layout, x S_R."""
    cat = np.concatenate([_w_tiles(Wi), _w_tiles(Wh)], axis=1) * S_R
    NT, NJ = cat.shape[:2]
    NQ = NJ // QT
    return np.ascontiguousarray(
        _to_bf16(cat.reshape(NT, NQ, QT, P, P).transpose(0, 1, 3, 2, 4))
    )


def _pack_w_fp8(W_ir, W_hr, W_hz, W_hh):
    """-> (NT, P, PAIRS, 2, P) e4m3 slab: 12 r-pairs + NPZ z + NPH h, x S_R."""
    KH = W_hr.shape[1] // P
    catr = np.concatenate([_w_tiles(W_ir), _w_tiles(W_hr)], axis=1)
    NT, NJ = catr.shape[:2]
    blocks = [catr.reshape(NT, NJ // 2, 2, P, P)]
    tz = _w_tiles(W_hz)                          # (NT, KH, p, m)
    blocks.append(tz[:, KH - 2 * NPZ :].reshape(NT, NPZ, 2, P, P))
    th = _w_tiles(W_hh)
    blocks.append(th[:, KH - 2 * NPH :].reshape(NT, NPH, 2, P, P))
    cat = np.concatenate(blocks, axis=1) * S_R   # (NT, PAIRS, 2, p, m)
    return np.ascontiguousarray(_to_e4m3(cat.transpose(0, 3, 1, 2, 4)))


def _pack_acts(a):
    """(Bc, D) -> (P, D//P, Bc) with [p, t, b] = a[b, t*P + p]."""
    Bc, D = a.shape
    return np.ascontiguousarray(a.T.reshape(D // P, P, Bc).transpose(1, 0, 2))


def run(x_t, h_prev, W_ir, W_iz, W_ih, W_hr, W_hz, W_hh, b_r, b_z, b_h,
        trace=False):
    from concourse.bass_utils import run_bass_kernel_spmd

    x_t = np.asarray(x_t, dtype=np.float32)
    h_prev = np.asarray(h_prev, dtype=np.float32)
    B, IN = x_t.shape
    H = h_prev.shape[1]
    assert B % N_CORES == 0
    Bc = B // N_CORES
    NT = H // P

    key = (Bc, IN, H)
    if key not in _PROG_CACHE:
        _PROG_CACHE[key] = build_program(Bc, IN, H)
    nc = _PROG_CACHE[key]

    f32 = np.float32
    wr = _pack_w_fp8(np.asarray(W_ir, f32), np.asarray(W_hr, f32),
                     np.asarray(W_hz, f32), np.asarray(W_hh, f32))
    wz = _pack_w_bf16(np.asarray(W_iz, f32), np.asarray(W_hz, f32))
    wh = _pack_w_bf16(np.asarray(W_ih, f32), np.asarray(W_hh, f32))
    bias = np.ascontiguousarray(
        np.stack(
            [np.asarray(b_r, f32), np.asarray(b_z, f32),
             np.asarray(b_h, f32)], axis=-1
        ).reshape(NT, P, 3).transpose(1, 0, 2).reshape(P, NT * 3)
    )

    in_maps = []
    for c in range(N_CORES):
        rows = slice(c * Bc, (c + 1) * Bc)
        xp = _pack_acts(x_t[rows])
        hp = _pack_acts(h_prev[rows])
        in_maps.append({
            "x8": _to_e4m3(xp), "h8": _to_e4m3(hp),
            "xb": _to_bf16(xp), "hb": _to_bf16(hp),
            "wr": wr, "wz": wz, "wh": wh, "bias": bias,
        })

    kw = {}
    if trace:
        kw = dict(trace=True, trace_cores=[0])
    res = run_bass_kernel_spmd(nc, in_maps, core_ids=list(range(N_CORES)), **kw)

    outs = []
    for c in range(N_CORES):
        o = np.asarray(res.results[c]["out"]).astype(np.float32)  # (NT, P, Bc)
        outs.append(o.reshape(H, Bc).T)                           # (Bc, H)
    full = np.concatenate(outs, axis=0).astype(np.float32)
    return (full, res) if trace else full


def kernel(**inputs):
    return run(**inputs)


# revision 14
# speedup vs baseline: 1.1537x; 1.0513x over previous
"""Bass/Trainium2 kernel for a fused GRU cell.

  r   = sigmoid(x @ W_ir.T + h @ W_hr.T + b_r)
  z   = sigmoid(x @ W_iz.T + h @ W_hz.T + b_z)
  g   = tanh  (x @ W_ih.T + (r*h) @ W_hh.T + b_h)
  h_t = (1-z)*h + z*g

Sharding: data-parallel over the batch (8192 -> 1024 rows per core on 8
NeuronCores), weights replicated, no collectives.

Mixed precision (numpy-simulated exactly: rel err 1.79e-2 vs 2e-2 budget;
HW matches the sim to ~1e-6 because inputs/reference are deterministic):
  - r gate: entirely fp8 e4m3 DoubleRow matmuls (2 k-tiles per 213ns MM =
    2x PE rate). r's quantization error washes out through the (r*h) @
    W_hh contraction, unlike z / h-tilde whose errors hit the output
    directly.
  - z gate: bf16 except the last h k-pair (tiles 14,15) in fp8-DR.
  - h-tilde: bf16 except the last five rh k-pairs (tiles 6..15) in fp8-DR
    (tanh saturation + the z<1 blend damp its quantization error, so it
    tolerates far more fp8 than z).
  - ALL gate weights are pre-scaled by 64 on host (exact in bf16, puts the
    fp8 weights in e4m3's normal range); every activation applies
    scale=1/64. Biases stay unscaled (activation computes f(x*scale+b)).
  - h_t is stored bf16 and upcast on host.

Layout is transposed ([hidden, batch], hidden on SBUF partitions) so biases
are per-partition scalars and all DMAs are contiguous. All fp8 weights are
preloaded into a resident SBUF tile; DMA triggers cost ~700ns each on the
issuing sequencer, so loads are batched, ordered in exact consumption
order on the sync queue, and hb stays chunked because the per-h-tile rhb
muls (which recycle the PSUM pool gating phase R) consume it tile by tile.
"""

import sys

for _p in ("/opt/trn_rl_repo", "/root/.axon_site/_ro/trn_rl_repo"):
    if _p not in sys.path:
        sys.path.append(_p)

import numpy as np

P = 128          # SBUF partitions
BC = 512         # PSUM bank free dim (fp32)
N_CORES = 8
S_R = 64.0       # weight prescale (undone in activation scale)
NPZ = 1          # fp8 k-pairs in the z gate (h-side tail)
NPH = 5          # fp8 k-pairs in the h-tilde gate (rh-side tail)
QTZ = 11         # bf16 z-weight k-tiles per DMA slab   (2 slabs = 22)
QTH = 7          # bf16 h-weight k-tiles per DMA slab   (2 slabs = 14)

_PROG_CACHE = {}


def build_program(Bc, IN, H):
    """Build the per-core SPMD Bass program (identical on all cores)."""
    from contextlib import ExitStack

    from concourse import bacc, bass, mybir, tile
    from concourse.dt import dt

    KI, KH, NT = IN // P, H // P, H // P
    NJ = KI + KH                 # contraction k-tiles per gate per h-tile
    NJP = NJ // 2                # r-gate fp8 pair-tiles
    PAIRS = NJP + NPZ + NPH      # fp8 pair-tiles per h-tile slab (r + z + h)
    NJZ = NJ - 2 * NPZ           # bf16 k-tiles in z gate
    NJH = NJ - 2 * NPH           # bf16 k-tiles in h-tilde gate
    NRB = KH - 2 * NPH           # rh tiles kept in bf16
    NB = Bc // BC
    assert NJZ == 2 * QTZ and NJH == 2 * QTH
    f32, bf16, f8 = dt.float32, dt.bfloat16, dt.float8e4
    SIG = mybir.ActivationFunctionType.Sigmoid
    TANH = mybir.ActivationFunctionType.Tanh
    DR = mybir.MatmulPerfMode.DoubleRow

    nc = bacc.Bacc("TRN2", debug=False)
    x8_d = nc.declare_dram_parameter("x8", [P, KI, Bc], f8, False)
    h8_d = nc.declare_dram_parameter("h8", [P, KH, Bc], f8, False)
    xb_d = nc.declare_dram_parameter("xb", [P, KI, Bc], bf16, False)
    hb_d = nc.declare_dram_parameter("hb", [P, KH, Bc], bf16, False)
    wr_d = nc.declare_dram_parameter("wr", [NT, P, PAIRS, 2, P], f8, False)
    wz_d = nc.declare_dram_parameter("wz", [NT, 2, P, QTZ, P], bf16, False)
    wh_d = nc.declare_dram_parameter("wh", [NT, 2, P, QTH, P], bf16, False)
    b_d = nc.declare_dram_parameter("bias", [P, NT * 3], f32, False)
    out_d = nc.declare_dram_parameter("out", [NT, P, Bc], bf16, True)

    with ExitStack() as ctx:
        tc = ctx.enter_context(tile.TileContext(nc))
        res = ctx.enter_context(tc.tile_pool(name="res", bufs=1))
        wpz = ctx.enter_context(tc.tile_pool(name="wpz", bufs=3))
        wph = ctx.enter_context(tc.tile_pool(name="wph", bufs=3))
        pp = ctx.enter_context(
            tc.tile_pool(name="pp", bufs=4, space=bass.MemorySpace.PSUM)
        )
        op = ctx.enter_context(tc.tile_pool(name="op", bufs=3))
        zp = ctx.enter_context(tc.tile_pool(name="zp", bufs=3))

        x8 = res.tile([P, KI, Bc], f8, tag="x8")
        h8 = res.tile([P, KH, Bc], f8, tag="h8")
        xb = res.tile([P, KI, Bc], bf16, tag="xb")
        hb = res.tile([P, KH, Bc], bf16, tag="hb")
        rhb = res.tile([P, NRB, Bc], bf16, tag="rhb")
        rh8 = res.tile([P, 2 * NPH, Bc], f8, tag="rh8")
        wr_all = res.tile([P, NT * PAIRS, 2, P], f8, tag="wr")
        bias = res.tile([P, NT * 3], f32, tag="bias")

        def wr_slab(hti):
            o = hti * PAIRS
            nc.sync.dma_start(out=wr_all[:, o : o + PAIRS], in_=wr_d[hti])

        nc.sync.dma_start(out=bias[:], in_=b_d[:])
        nc.sync.dma_start(out=x8[:, : KI // 2], in_=x8_d[:, : KI // 2])
        wr_slab(0)
        nc.sync.dma_start(out=x8[:, KI // 2 :], in_=x8_d[:, KI // 2 :])
        # slabs 1-3 ahead of h8: their x-side matmuls cover the h8 wait
        for i in range(1, 4):
            wr_slab(i)
        nc.sync.dma_start(out=h8[:], in_=h8_d[:])
        # interleave r slabs (consumed 1 per 5.2us) with hb chunks (1 per
        # 10.3us via the rhb muls) so neither starves the other
        for i in range(4, NT):
            wr_slab(i)
            t = 2 * (i - 4)
            if t < KH:
                nc.sync.dma_start(out=hb[:, t : t + 2, :], in_=hb_d[:, t : t + 2])
        for t in range(2 * (NT - 4), KH, 2):
            nc.sync.dma_start(out=hb[:, t : t + 2, :], in_=hb_d[:, t : t + 2])
        # bf16 x (phase ZH): behind all phase-R data, ahead of ZH slabs
        nc.sync.dma_start(out=xb[:], in_=xb_d[:])

        # ---- phase R: r = sigmoid((gi_r + gh_r)/S + b_r); rh = r * h ----
        for hti in range(NT):
            ps = pp.tile([P, Bc], f32, tag="ps")
            for pj in range(NJP):
                mov = (
                    x8[:, 2 * pj : 2 * pj + 2, :]
                    if pj < KI // 2
                    else h8[:, 2 * pj - KI : 2 * pj - KI + 2, :]
                )
                for bc in range(NB):
                    sl = slice(bc * BC, (bc + 1) * BC)
                    nc.tensor.matmul(
                        ps[:, sl],
                        wr_all[:, hti * PAIRS + pj],
                        mov[:, :, sl],
                        start=(pj == 0),
                        stop=(pj == NJP - 1),
                        perf_mode=DR,
                        skip_group_check=True,
                    )
            for bc in range(NB):
                sl = slice(bc * BC, (bc + 1) * BC)
                nc.scalar.activation(
                    ps[:, sl], ps[:, sl], SIG,
                    bias=bias[:, hti * 3 : hti * 3 + 1], scale=1.0 / S_R,
                )
                if hti < NRB:
                    nc.vector.tensor_mul(rhb[:, hti, sl], ps[:, sl], hb[:, hti, sl])
                else:
                    nc.vector.tensor_mul(
                        rh8[:, hti - NRB, sl], ps[:, sl], hb[:, hti, sl]
                    )

        def gate(ps, w_d, wpool, qt, hti, srch, pair0, pairs_mov):
            # bf16 part: ps[:, bc] += sum_{j<2*qt} W_tile[j].T @ moving[j]
            for q in range(2):
                slab = wpool.tile([P, qt, P], bf16, tag="w")
                nc.sync.dma_start(out=slab[:], in_=w_d[hti, q])
                for jj in range(qt):
                    j = q * qt + jj
                    mov = xb[:, j, :] if j < KI else srch[:, j - KI, :]
                    for bc in range(NB):
                        sl = slice(bc * BC, (bc + 1) * BC)
                        nc.tensor.matmul(
                            ps[:, sl],
                            slab[:, jj],
                            mov[:, sl],
                            start=(j == 0),
                            stop=False,
                            skip_group_check=True,
                        )
            # fp8-DR tail pairs (weights live in the resident wr_all slab)
            for i, pmov in enumerate(pairs_mov):
                for bc in range(NB):
                    sl = slice(bc * BC, (bc + 1) * BC)
                    nc.tensor.matmul(
                        ps[:, sl],
                        wr_all[:, hti * PAIRS + pair0 + i],
                        pmov[:, :, sl],
                        start=False,
                        stop=(i == len(pairs_mov) - 1),
                        perf_mode=DR,
                        skip_group_check=True,
                    )

        # ---- phase ZH: z, g, h_t = h + z*(g - h) ----
        for hti in range(NT):
            psz = pp.tile([P, Bc], f32, tag="ps")
            gate(psz, wz_d, wpz, QTZ, hti, hb, NJP,
                 [h8[:, KH - 2 * NPZ + 2 * i : KH - 2 * NPZ + 2 * i + 2, :]
                  for i in range(NPZ)])
            psh = pp.tile([P, Bc], f32, tag="ps")
            gate(psh, wh_d, wph, QTH, hti, rhb, NJP + NPZ,
                 [rh8[:, 2 * i : 2 * i + 2, :] for i in range(NPH)])
            # last h-tile: finer chunks shorten the serial act->DVE->store
            # chain that forms the kernel tail
            cw = BC // 2 if hti == NT - 1 else BC
            for c in range(Bc // cw):
                sl = slice(c * cw, (c + 1) * cw)
                # z straight into SBUF (DVE may read only one PSUM operand)
                zs = zp.tile([P, BC], f32, tag="zs")
                nc.scalar.activation(
                    zs[:, :cw], psz[:, sl], SIG,
                    bias=bias[:, hti * 3 + 1 : hti * 3 + 2], scale=1.0 / S_R,
                )
                nc.scalar.activation(
                    psh[:, sl], psh[:, sl], TANH,
                    bias=bias[:, hti * 3 + 2 : hti * 3 + 3], scale=1.0 / S_R,
                )
                nc.vector.tensor_sub(psh[:, sl], psh[:, sl], hb[:, hti, sl])
                nc.vector.tensor_mul(psh[:, sl], zs[:, :cw], psh[:, sl])
                o = op.tile([P, BC], bf16, tag="o")
                nc.vector.tensor_add(o[:, :cw], psh[:, sl], hb[:, hti, sl])
                nc.gpsimd.dma_start(out=out_d[hti, :, sl], in_=o[:, :cw])

    nc.compile()
    return nc


def _to_e4m3(a):
    import ml_dtypes

    return np.clip(a, -240.0, 240.0).astype(ml_dtypes.float8_e4m3)


def _to_bf16(a):
    import ml_dtypes

    return a.astype(ml_dtypes.bfloat16)


def _w_tiles(W):
    """(H, K) -> (NT, K//P, p, m) of 128x128 W.T blocks.

    t[hti, j][p, m] = W[hti*P + m, j*P + p]
    """
    H, K = W.shape
    return W.reshape(H // P, P, K // P, P).transpose(0, 2, 3, 1)


def _pack_w_bf16(Wi, Wh, qt):
    """-> (NT, 2, P, qt, P) bf16 DMA-slab layout (first 2*qt k-tiles), xS."""
    cat = np.concatenate([_w_tiles(Wi), _w_tiles(Wh)], axis=1)[:, : 2 * qt] * S_R
    NT = cat.shape[0]
    return np.ascontiguousarray(
        _to_bf16(cat.reshape(NT, 2, qt, P, P).transpose(0, 1, 3, 2, 4))
    )


def _pack_w_fp8(W_ir, W_hr, W_hz, W_hh):
    """-> (NT, P, PAIRS, 2, P) e4m3 slab: r-pairs + NPZ z + NPH h, x S_R."""
    KH = W_hr.shape[1] // P
    catr = np.concatenate([_w_tiles(W_ir), _w_tiles(W_hr)], axis=1)
    NT, NJ = catr.shape[:2]
    blocks = [catr.reshape(NT, NJ // 2, 2, P, P)]
    tz = _w_tiles(W_hz)                          # (NT, KH, p, m)
    blocks.append(tz[:, KH - 2 * NPZ :].reshape(NT, NPZ, 2, P, P))
    th = _w_tiles(W_hh)
    blocks.append(th[:, KH - 2 * NPH :].reshape(NT, NPH, 2, P, P))
    cat = np.concatenate(blocks, axis=1) * S_R   # (NT, PAIRS, 2, p, m)
    return np.ascontiguousarray(_to_e4m3(cat.transpose(0, 3, 1, 2, 4)))


def _pack_acts(a):
    """(Bc, D) -> (P, D//P, Bc) with [p, t, b] = a[b, t*P + p]."""
    Bc, D = a.shape
    return np.ascontiguousarray(a.T.reshape(D // P, P, Bc).transpose(1, 0, 2))


def run(x_t, h_prev, W_ir, W_iz, W_ih, W_hr, W_hz, W_hh, b_r, b_z, b_h,
        trace=False):
    from concourse.bass_utils import run_bass_kernel_spmd

    x_t = np.asarray(x_t, dtype=np.float32)
    h_prev = np.asarray(h_prev, dtype=np.float32)
    B, IN = x_t.shape
    H = h_prev.shape[1]
    assert B % N_CORES == 0
    Bc = B // N_CORES
    NT = H // P

    key = (Bc, IN, H)
    if key not in _PROG_CACHE:
        _PROG_CACHE[key] = build_program(Bc, IN, H)
    nc = _PROG_CACHE[key]

    f32 = np.float32
    wr = _pack_w_fp8(np.asarray(W_ir, f32), np.asarray(W_hr, f32),
                     np.asarray(W_hz, f32), np.asarray(W_hh, f32))
    wz = _pack_w_bf16(np.asarray(W_iz, f32), np.asarray(W_hz, f32), QTZ)
    wh = _pack_w_bf16(np.asarray(W_ih, f32), np.asarray(W_hh, f32), QTH)
    bias = np.ascontiguousarray(
        np.stack(
            [np.asarray(b_r, f32), np.asarray(b_z, f32),
             np.asarray(b_h, f32)], axis=-1
        ).reshape(NT, P, 3).transpose(1, 0, 2).reshape(P, NT * 3)
    )

    in_maps = []
    for c in range(N_CORES):
        rows = slice(c * Bc, (c + 1) * Bc)
        xp = _pack_acts(x_t[rows])
        hp = _pack_acts(h_prev[rows])
        in_maps.append({
            "x8": _to_e4m3(xp), "h8": _to_e4m3(hp),
            "xb": _to_bf16(xp), "hb": _to_bf16(hp),
            "wr": wr, "wz": wz, "wh": wh, "bias": bias,
        })

    kw = {}
    if trace:
        kw = dict(trace=True, trace_cores=[0])
    res = run_bass_kernel_spmd(nc, in_maps, core_ids=list(range(N_CORES)), **kw)

    outs = []
    for c in range(N_CORES):
        o = np.asarray(res.results[c]["out"]).astype(np.float32)  # (NT, P, Bc)
        outs.append(o.reshape(H, Bc).T)                           # (Bc, H)
    full = np.concatenate(outs, axis=0).astype(np.float32)
    return (full, res) if trace else full


def kernel(**inputs):
    return run(**inputs)
